# revision 3
# baseline (speedup 1.0000x reference)
"""Causal self-attention (B=4, S=2048, E=2048, H=16, D=128) on 8 TRN2 cores.

Sharding: batch (4-way) x head-halves (2-way) -> 8 cores.
Core c handles batch b = c//2, heads p*8..p*8+8 where p = c%2.

Per-core kernel (single NEFF, SPMD):
  Phase 1: QKV projection.  x^T resident in SBUF [E,S]; per head computes
           q^T, k^T in [D,S] layout (matmul lhsT = W columns, rhs = x^T) and
           v in [S,D] layout (lhsT = x^T tile, rhs = W_v columns).
           q^T -> DRAM scratch, k^T -> output, v -> output.
  Phase 2: attention + proj, in two head-groups of 4.
           Scores computed transposed: ST[k,q] = (kT chunk).T-free vs qT.
           exp on scalar engine (no max subtraction needed: scores ~ N(0,1)),
           row sums via ones-vector matmul, y^T via lhsT=v rhs=P^T,
           normalization via PE outer-product broadcast of 1/sums,
           out = y @ Wproj accumulated over the 4 heads in PSUM.
Host: shards inputs, sums the 4 partial outs per batch, fixes up k/v biases.

Matmul inputs use float32r (single-pass PE, mantissa rounded to 12 bits)
unless CK_MM_DT=f32 (2-pass full fp32, ~4x slower).
"""

import math
import os
import sys

for _p in ("/opt/trn_rl_repo",):
    if os.path.isdir(_p) and _p not in sys.path:
        sys.path.append(_p)

import numpy as np

import concourse.bacc as bacc
import concourse.mybir as mybir
import concourse.tile as tile
from concourse.bass_utils import run_bass_kernel_spmd

B, S, E, H = 4, 2048, 2048, 16
D = E // H            # 128
P = 128               # partitions
HPC = H // 2          # heads per core = 8
NE = E // P           # 16 e-chunks
NS = S // P           # 16 s-chunks
STQ = 512             # phase-2 q tile width
NR = S // STQ         # 4 q tiles
NORM = 1.0 / math.sqrt(D)
NEG = -1.0e30

F32 = mybir.dt.float32
F32R = mybir.dt.float32r

# matmul input dtype: float32r = full-rate single-pass (12-bit mantissa),
# float32 = 4x slower 2-pass full precision.
MMDT = {"f32": F32, "f32r": F32R}[os.environ.get("CK_MM_DT", "f32r")]


def _build_program():
    nc = bacc.Bacc("TRN2", target_bir_lowering=False, debug=False)
    Exp = mybir.ActivationFunctionType.Exp
    mm = nc.tensor.matmul

    with tile.TileContext(nc) as tc:
        with tc.tile_pool(name="dram", bufs=1, space="DRAM") as dram:
            def din(name, shape, dt=MMDT):
                return dram.tile(shape, dt, kind="ExternalInput", name=name,
                                 uniquify=False)

            def dout(name, shape, dt=F32):
                return dram.tile(shape, dt, kind="ExternalOutput", name=name,
                                 uniquify=False)

            xT_d = din("xT", [E, S])
            wq_d = din("wq", [E, HPC * D])
            wk_d = din("wk", [E, HPC * D])
            wv_d = din("wv", [E, HPC * D])
            wp_d = din("wp", [HPC * D, E])
            bq_d = din("bq", [P, HPC], F32)
            # k^T / v outputs carry matmul-rounded values (valid fp32 bits)
            kT_d = dout("kT_out", [HPC, P, S], MMDT)
            v_d = dout("v_out", [S, HPC * D], MMDT)
            o_d = [dout("out_pA", [S, E]), dout("out_pB", [S, E])]
            qT_d = dram.tile([HPC, P, S], MMDT, kind="Internal", name="qT_s",
                             uniquify=False)

            # ---------------- constants ----------------
            with tc.tile_pool(name="const", bufs=1) as cp:
                masks = []
                for m in range(4):
                    mk = cp.tile([P, STQ], F32, name=f"mask{m}", tag=f"mask{m}")
                    nc.gpsimd.memset(mk, 0.0)
                    # ST layout [k, q]: keep where q - k - 128*m >= 0
                    nc.gpsimd.affine_select(
                        out=mk, in_=mk,
                        compare_op=mybir.AluOpType.is_ge,
                        fill=NEG, base=-P * m,
                        channel_multiplier=-1, pattern=[[1, STQ]],
                    )
                    masks.append(mk)
                ones_col_f = cp.tile([P, 1], F32, name="ones_col_f", tag="oncf")
                nc.gpsimd.memset(ones_col_f, 1.0)
                ones_col = cp.tile([P, 1], MMDT, name="ones_col", tag="onc")
                nc.scalar.copy(ones_col, ones_col_f)
                ones_row_f = cp.tile([1, P], F32, name="ones_row_f", tag="onrf")
                nc.gpsimd.memset(ones_row_f, 1.0)
                ones_row = cp.tile([1, P], MMDT, name="ones_row", tag="onr")
                nc.scalar.copy(ones_row, ones_row_f)
                bq_sb = cp.tile([P, HPC], F32, name="bq_sb", tag="bq")
                nc.sync.dma_start(bq_sb, bq_d)

                # ---------------- phase 1: QKV ----------------
                with tc.tile_pool(name="p1", bufs=1) as p1, \
                     tc.tile_pool(name="wstream", bufs=4) as ws, \
                     tc.tile_pool(name="wvres", bufs=NE) as wvp, \
                     tc.tile_pool(name="ev", bufs=4) as evp, \
                     tc.tile_pool(name="psA", bufs=5, space="PSUM") as psA, \
                     tc.tile_pool(name="psV", bufs=3, space="PSUM") as psV:

                    xt = p1.tile([P, NE, S], MMDT, name="xt", tag="xt")
                    for c in range(NE):
                        nc.sync.dma_start(xt[:, c, :], xT_d[c * P:(c + 1) * P, :])

                    # q^T and k^T per head
                    for h in range(HPC):
                        for which, w_d, dst, bias in (
                            ("q", wq_d, qT_d, bq_sb),
                            ("k", wk_d, kT_d, None),
                        ):
                            ps = [psA.tile([P, 512], F32,
                                           name=f"ps_{which}{h}_{n}", tag="ps")
                                  for n in range(4)]
                            for c in range(NE):
                                wt = ws.tile([P, D], MMDT,
                                             name=f"w_{which}{h}_{c}", tag="w")
                                nc.sync.dma_start(
                                    wt, w_d[c * P:(c + 1) * P, h * D:(h + 1) * D])
                                for n in range(4):
                                    mm(ps[n], lhsT=wt,
                                       rhs=xt[:, c, n * 512:(n + 1) * 512],
                                       start=(c == 0), stop=(c == NE - 1))
                            for n in range(4):
                                ev = evp.tile([P, 512], MMDT,
                                              name=f"ev_{which}{h}_{n}", tag="ev")
                                if bias is not None:
                                    nc.scalar.activation(
                                        ev, ps[n],
                                        mybir.ActivationFunctionType.Identity,
                                        bias=bias[:, h:h + 1], scale=1.0)
                                else:
                                    nc.scalar.copy(ev, ps[n])
                                nc.sync.dma_start(
                                    dst[h, :, n * 512:(n + 1) * 512], ev)

                    # v in [S, 4*D] groups of 4 heads
                    for g4 in range(2):
                        wvt = []
                        for c in range(NE):
                            wvc = wvp.tile([P, 512], MMDT,
                                           name=f"wv_{g4}_{c}", tag="wv")
                            nc.sync.dma_start(
                                wvc, wv_d[c * P:(c + 1) * P,
                                          g4 * 512:(g4 + 1) * 512])
                            wvt.append(wvc)
                        for st in range(NS):
                            pv = psV.tile([P, 512], F32,
                                          name=f"pv_{g4}_{st}", tag="pv")
                            for c in range(NE):
                                mm(pv, lhsT=xt[:, c, st * P:(st + 1) * P],
                                   rhs=wvt[c], start=(c == 0), stop=(c == NE - 1))
                            ev = evp.tile([P, 512], MMDT,
                                          name=f"evv_{g4}_{st}", tag="ev")
                            nc.scalar.copy(ev, pv)
                            nc.sync.dma_start(
                                v_d[st * P:(st + 1) * P,
                                    g4 * 512:(g4 + 1) * 512], ev)

                # ---------------- phase 2: attention + proj ----------------
                for g in range(2):
                    with tc.tile_pool(name=f"kv{g}", bufs=1) as kv, \
                         tc.tile_pool(name=f"wpp{g}", bufs=1) as wpp, \
                         tc.tile_pool(name=f"qt{g}", bufs=3) as qtp, \
                         tc.tile_pool(name=f"pt{g}", bufs=4) as ptp, \
                         tc.tile_pool(name=f"tmp{g}", bufs=2) as tmpp, \
                         tc.tile_pool(name=f"bc{g}", bufs=2) as bcp, \
                         tc.tile_pool(name=f"rc{g}", bufs=2) as rcp, \
                         tc.tile_pool(name=f"ytn{g}", bufs=2) as ytnp, \
                         tc.tile_pool(name=f"ob{g}", bufs=3) as obp, \
                         tc.tile_pool(name=f"psT{g}", bufs=3, space="PSUM") as psT, \
                         tc.tile_pool(name=f"psY{g}", bufs=2, space="PSUM") as psY, \
                         tc.tile_pool(name=f"psS{g}", bufs=1, space="PSUM") as psS, \
                         tc.tile_pool(name=f"psO{g}", bufs=2, space="PSUM") as psO:

                        kt_sb, v_sb, wp_sb = [], [], []
                        for hh in range(4):
                            h = g * 4 + hh
                            kt = kv.tile([P, S], MMDT, name=f"kt{g}_{hh}",
                                         tag=f"kt{hh}")
                            for n in range(4):
                                nc.sync.dma_start(
                                    kt[:, n * 512:(n + 1) * 512],
                                    kT_d[h, :, n * 512:(n + 1) * 512])
                            kt_sb.append(kt)
                            vt = kv.tile([P, NS, D], MMDT, name=f"v{g}_{hh}",
                                         tag=f"v{hh}")
                            for j in range(NS):
                                nc.sync.dma_start(
                                    vt[:, j, :],
                                    v_d[j * P:(j + 1) * P, h * D:(h + 1) * D])
                            v_sb.append(vt)
                            wpt = wpp.tile([P, E], MMDT, name=f"wp{g}_{hh}",
                                           tag=f"wp{hh}")
                            for n in range(4):
                                nc.sync.dma_start(
                                    wpt[:, n * 512:(n + 1) * 512],
                                    wp_d[h * P:(h + 1) * P,
                                         n * 512:(n + 1) * 512])
                            wp_sb.append(wpt)

                        for r in range(NR):
                            ytn_tiles = []
                            for hh in range(4):
                                h = g * 4 + hh
                                qt = qtp.tile([P, STQ], MMDT,
                                              name=f"qt{g}_{r}_{hh}", tag="qt")
                                nc.sync.dma_start(
                                    qt, qT_d[h, :, r * STQ:(r + 1) * STQ])
                                nj = 4 * (r + 1)
                                yt_ps = psY.tile([P, STQ], F32,
                                                 name=f"yt{g}_{r}_{hh}", tag="yt")
                                sm_ps = psS.tile([1, STQ], F32,
                                                 name=f"sm{g}_{r}_{hh}", tag="sm")
                                for j in range(nj):
                                    st_ps = psT.tile(
                                        [P, STQ], F32,
                                        name=f"st{g}_{r}_{hh}_{j}", tag="st")
                                    mm(st_ps, lhsT=kt_sb[hh][:, j * P:(j + 1) * P],
                                       rhs=qt, start=True, stop=True)
                                    pt = ptp.tile([P, STQ], MMDT,
                                                  name=f"pt{g}_{r}_{hh}_{j}",
                                                  tag="pt")
                                    if j >= nj - 4:
                                        tmp = tmpp.tile(
                                            [P, STQ], F32,
                                            name=f"tm{g}_{r}_{hh}_{j}", tag="tmp")
                                        nc.vector.tensor_add(
                                            tmp, st_ps, masks[j - (nj - 4)])
                                        nc.scalar.activation(pt, tmp, Exp,
                                                             scale=NORM)
                                    else:
                                        nc.scalar.activation(pt, st_ps, Exp,
                                                             scale=NORM)
                                    mm(sm_ps, lhsT=ones_col, rhs=pt,
                                       start=(j == 0), stop=(j == nj - 1))
                                    mm(yt_ps, lhsT=v_sb[hh][:, j, :], rhs=pt,
                                       start=(j == 0), stop=(j == nj - 1))
                                rc = rcp.tile([1, STQ], F32,
                                              name=f"rc{g}_{r}_{hh}", tag="rc")
                                nc.vector.reciprocal(rc, sm_ps)
                                rcr = rcp.tile([1, STQ], MMDT,
                                               name=f"rcr{g}_{r}_{hh}", tag="rcr")
                                nc.scalar.copy(rcr, rc)
                                bc_ps = psT.tile([P, STQ], F32,
                                                 name=f"bcp{g}_{r}_{hh}", tag="st")
                                mm(bc_ps, lhsT=ones_row, rhs=rcr,
                                   start=True, stop=True)
                                bc_sb = bcp.tile([P, STQ], F32,
                                                 name=f"bc{g}_{r}_{hh}", tag="bc")
                                nc.scalar.copy(bc_sb, bc_ps)
                                ytn = ytnp.tile([P, STQ], MMDT,
                                                name=f"ytn{g}_{r}_{hh}",
                                                tag=f"ytn{hh}")
                                nc.vector.tensor_mul(ytn, yt_ps, bc_sb)
                                ytn_tiles.append(ytn)

                            for n in range(4):
                                for qq in range(4):
                                    o_ps = psO.tile([P, 512], F32,
                                                    name=f"o{g}_{r}_{n}_{qq}",
                                                    tag="o")
                                    for hh in range(4):
                                        mm(o_ps,
                                           lhsT=ytn_tiles[hh][:, qq * P:(qq + 1) * P],
                                           rhs=wp_sb[hh][:, n * 512:(n + 1) * 512],
                                           start=(hh == 0), stop=(hh == 3))
                                    o_sb = obp.tile([P, 512], F32,
                                                    name=f"ob{g}_{r}_{n}_{qq}",
                                                    tag="ob")
                                    nc.scalar.copy(o_sb, o_ps)
                                    nc.sync.dma_start(
                                        o_d[g][r * STQ + qq * P:
                                               r * STQ + (qq + 1) * P,
                                               n * 512:(n + 1) * 512], o_sb)

    nc.compile()
    return nc


_NC = None


def _get_program():
    global _NC
    if _NC is None:
        _NC = _build_program()
    return _NC


def _maybe_install_trace_shim():
    """Provide antenv.axon_hooks (NTFF profiling) if the image lacks it."""
    import types
    if "antenv.axon_hooks" in sys.modules:
        return
    try:
        from trn_agent_boot.trn_boot import _ntff_profile_via_ctypes
        hook = _ntff_profile_via_ctypes("/opt/axon/libaxon_pjrt.so")
    except Exception:
        return
    mod = types.ModuleType("antenv.axon_hooks")
    mod.get_axon_ntff_profile_hook = lambda: hook
    mod.set_axon_ntff_profile_hook = lambda h: None
    sys.modules["antenv.axon_hooks"] = mod


def kernel(x, W_attn, b_attn, W_proj, b_proj):
    x = np.ascontiguousarray(np.asarray(x, dtype=np.float32))
    W_attn = np.ascontiguousarray(np.asarray(W_attn, dtype=np.float32))
    b_attn = np.ascontiguousarray(np.asarray(b_attn, dtype=np.float32))
    W_proj = np.ascontiguousarray(np.asarray(W_proj, dtype=np.float32))
    b_proj = np.ascontiguousarray(np.asarray(b_proj, dtype=np.float32))

    nc = _get_program()

    # per-parity weight shards (heads p*8 .. p*8+8)
    shards = []
    for p in range(2):
        cs = slice(p * HPC * D, (p + 1) * HPC * D)
        shards.append({
            "wq": np.ascontiguousarray(W_attn[:, 0 * E:1 * E][:, cs]),
            "wk": np.ascontiguousarray(W_attn[:, 1 * E:2 * E][:, cs]),
            "wv": np.ascontiguousarray(W_attn[:, 2 * E:3 * E][:, cs]),
            "wp": np.ascontiguousarray(W_proj[cs, :]),
            "bq": np.ascontiguousarray(
                b_attn[0 * E:1 * E][cs].reshape(HPC, D).T),
        })
    xTs = [np.ascontiguousarray(x[b].T) for b in range(B)]

    in_maps = []
    for core in range(8):
        b, p = core // 2, core % 2
        m = {"xT": xTs[b]}
        m.update(shards[p])
        in_maps.append(m)

    trace = bool(os.environ.get("CK_TRACE"))
    if trace:
        _maybe_install_trace_shim()
    res = run_bass_kernel_spmd(nc, in_maps, core_ids=list(range(8)),
                               trace=trace)
    if trace:
        kernel.last_exec_time_ns = res.exec_time_ns
        kernel.last_trace = res.instructions_and_trace

    # ------- host-side gather -------
    b_k = b_attn[1 * E:2 * E]
    b_v = b_attn[2 * E:3 * E]

    out = np.empty((B, S, E), dtype=np.float32)
    k_full = np.empty((B, H, S, D), dtype=np.float32)
    v_full = np.empty((B, H, S, D), dtype=np.float32)
    bias_out = (b_v @ W_proj + b_proj).astype(np.float32)

    for bi in range(B):
        r0, r1 = res.results[2 * bi], res.results[2 * bi + 1]
        out[bi] = (r0["out_pA"] + r0["out_pB"] + r1["out_pA"] + r1["out_pB"]
                   + bias_out[None, :])
        for p, r in ((0, r0), (1, r1)):
            for j in range(HPC):
                h = p * HPC + j
                k_full[bi, h] = (r["kT_out"][j].T
                                 + b_k[h * D:(h + 1) * D][None, :])
                v_full[bi, h] = (r["v_out"][:, j * D:(j + 1) * D]
                                 + b_v[h * D:(h + 1) * D][None, :])

    return out, k_full, v_full


# revision 4
# speedup vs baseline: 1.1303x; 1.1303x over previous
"""Causal self-attention (B=4, S=2048, E=2048, H=16, D=128) on 8 TRN2 cores.

Sharding: batch (4-way) x head-halves (2-way) -> 8 cores.
Core c handles batch b = c//2, heads p*8..p*8+8 where p = c%2.

Per-core kernel (single NEFF, SPMD):
  Phase 1: QKV projection.  x^T resident in SBUF [E,S]; per head computes
           q^T, k^T in [D,S] layout (matmul lhsT = W columns, rhs = x^T) and
           v in [S,D] layout (lhsT = x^T tile, rhs = W_v columns).
           q^T -> DRAM scratch, k^T -> output, v -> output.
  Phase 2: attention + proj in one pass; k^T resident for all 8 heads,
           v / Wproj / q^T streamed.
           Scores computed transposed: ST[k,q] = (kT chunk) lhsT vs qT rhs.
           exp on scalar engine (no max subtraction needed: scores ~ N(0,1)),
           row sums via ones-vector matmul, y^T via lhsT=v rhs=P^T,
           normalization via PE outer-product broadcast of 1/sums,
           out = y @ Wproj accumulated over all 8 heads in PSUM.
Host: shards inputs, sums the 2 partial outs per batch, fixes up k/v biases.

All DRAM tensors are laid out so every DMA transfer is a single contiguous
block (weights pre-packed on host into [head, chunk, 128, width] form).

Matmul inputs use float32r (single-pass PE, mantissa rounded to 12 bits)
unless CK_MM_DT=f32 (2-pass full fp32, ~4x slower).
"""

import math
import os
import sys

for _p in ("/opt/trn_rl_repo",):
    if os.path.isdir(_p) and _p not in sys.path:
        sys.path.append(_p)

import numpy as np

import concourse.bacc as bacc
import concourse.mybir as mybir
import concourse.tile as tile
from concourse.bass_utils import run_bass_kernel_spmd

B, S, E, H = 4, 2048, 2048, 16
D = E // H            # 128
P = 128               # partitions
HPC = H // 2          # heads per core = 8
NE = E // P           # 16 e-chunks
NS = S // P           # 16 s-chunks
STQ = 512             # phase-2 q tile width
NR = S // STQ         # 4 q tiles
NN = E // 512         # 4 output col chunks
NORM = 1.0 / math.sqrt(D)
NEG = -1.0e30

F32 = mybir.dt.float32
F32R = mybir.dt.float32r

# matmul input dtype: float32r = full-rate single-pass (12-bit mantissa),
# float32 = 4x slower 2-pass full precision.
MMDT = {"f32": F32, "f32r": F32R}[os.environ.get("CK_MM_DT", "f32r")]


def _build_program():
    nc = bacc.Bacc("TRN2", target_bir_lowering=False, debug=False)
    Exp = mybir.ActivationFunctionType.Exp
    mm = nc.tensor.matmul

    with tile.TileContext(nc) as tc:
        with tc.tile_pool(name="dram", bufs=1, space="DRAM") as dram:
            def din(name, shape, dt=MMDT):
                return dram.tile(shape, dt, kind="ExternalInput", name=name,
                                 uniquify=False)

            def dout(name, shape, dt=F32):
                return dram.tile(shape, dt, kind="ExternalOutput", name=name,
                                 uniquify=False)

            xT_d = din("xT", [E, S])
            wq_d = din("wq", [HPC, NE, P, D])
            wk_d = din("wk", [HPC, NE, P, D])
            wv_d = din("wv", [2, NE, P, 512])
            wp_d = din("wp", [HPC, NN, P, 512])
            bq_d = din("bq", [P, HPC], F32)
            # k^T / v outputs carry matmul-rounded values (valid fp32 bits)
            kT_d = dout("kT_out", [HPC, NR, P, 512], MMDT)
            v_d = dout("v_out", [HPC, NS, P, D], MMDT)
            o_d = dout("out_p", [S, E])
            qT_d = dram.tile([HPC, NR, P, 512], MMDT, kind="Internal",
                             name="qT_s", uniquify=False)

            # ---------------- constants ----------------
            with tc.tile_pool(name="const", bufs=1) as cp:
                masks = []
                for m in range(4):
                    mk = cp.tile([P, STQ], F32, name=f"mask{m}", tag=f"mask{m}")
                    nc.gpsimd.memset(mk, 0.0)
                    # ST layout [k, q]: keep where q - k - 128*m >= 0
                    nc.gpsimd.affine_select(
                        out=mk, in_=mk,
                        compare_op=mybir.AluOpType.is_ge,
                        fill=NEG, base=-P * m,
                        channel_multiplier=-1, pattern=[[1, STQ]],
                    )
                    masks.append(mk)
                ones_col_f = cp.tile([P, 1], F32, name="ones_col_f", tag="oncf")
                nc.gpsimd.memset(ones_col_f, 1.0)
                ones_col = cp.tile([P, 1], MMDT, name="ones_col", tag="onc")
                nc.scalar.copy(ones_col, ones_col_f)
                ones_row_f = cp.tile([1, P], F32, name="ones_row_f", tag="onrf")
                nc.gpsimd.memset(ones_row_f, 1.0)
                ones_row = cp.tile([1, P], MMDT, name="ones_row", tag="onr")
                nc.scalar.copy(ones_row, ones_row_f)
                bq_sb = cp.tile([P, HPC], F32, name="bq_sb", tag="bq")
                nc.sync.dma_start(bq_sb, bq_d)

                # ---------------- phase 1: QKV ----------------
                with tc.tile_pool(name="p1", bufs=1) as p1, \
                     tc.tile_pool(name="wstream", bufs=6) as ws, \
                     tc.tile_pool(name="wvres", bufs=NE) as wvp, \
                     tc.tile_pool(name="ev", bufs=6) as evp, \
                     tc.tile_pool(name="psA", bufs=5, space="PSUM") as psA, \
                     tc.tile_pool(name="psV", bufs=3, space="PSUM") as psV:

                    xt = p1.tile([P, NE, S], MMDT, name="xt", tag="xt")
                    for c in range(NE):
                        nc.sync.dma_start(xt[:, c, :], xT_d[c * P:(c + 1) * P, :])

                    # q^T and k^T per head
                    for h in range(HPC):
                        for which, w_d, dst, bias in (
                            ("q", wq_d, qT_d, bq_sb),
                            ("k", wk_d, kT_d, None),
                        ):
                            ps = [psA.tile([P, 512], F32,
                                           name=f"ps_{which}{h}_{n}", tag="ps")
                                  for n in range(4)]
                            for c in range(NE):
                                wt = ws.tile([P, D], MMDT,
                                             name=f"w_{which}{h}_{c}", tag="w")
                                nc.sync.dma_start(wt, w_d[h, c])
                                for n in range(4):
                                    mm(ps[n], lhsT=wt,
                                       rhs=xt[:, c, n * 512:(n + 1) * 512],
                                       start=(c == 0), stop=(c == NE - 1))
                            for n in range(4):
                                ev = evp.tile([P, 512], MMDT,
                                              name=f"ev_{which}{h}_{n}", tag="ev")
                                if bias is not None:
                                    nc.scalar.activation(
                                        ev, ps[n],
                                        mybir.ActivationFunctionType.Identity,
                                        bias=bias[:, h:h + 1], scale=1.0)
                                else:
                                    nc.scalar.copy(ev, ps[n])
                                nc.sync.dma_start(dst[h, n], ev)

                    # v in [S, 4*D] groups of 4 heads
                    for g4 in range(2):
                        wvt = []
                        for c in range(NE):
                            wvc = wvp.tile([P, 512], MMDT,
                                           name=f"wv_{g4}_{c}", tag="wv")
                            nc.sync.dma_start(wvc, wv_d[g4, c])
                            wvt.append(wvc)
                        for st in range(NS):
                            pv = psV.tile([P, 512], F32,
                                          name=f"pv_{g4}_{st}", tag="pv")
                            for c in range(NE):
                                mm(pv, lhsT=xt[:, c, st * P:(st + 1) * P],
                                   rhs=wvt[c], start=(c == 0), stop=(c == NE - 1))
                            ev = evp.tile([P, 512], MMDT,
                                          name=f"evv_{g4}_{st}", tag="ev")
                            nc.scalar.copy(ev, pv)
                            for hh in range(4):
                                h = g4 * 4 + hh
                                nc.sync.dma_start(
                                    v_d[h, st], ev[:, hh * D:(hh + 1) * D])

                # ---------------- phase 2: attention + proj ----------------
                with tc.tile_pool(name="ktp", bufs=1) as ktp, \
                     tc.tile_pool(name="vsp", bufs=8) as vsp, \
                     tc.tile_pool(name="wpp", bufs=16) as wpp, \
                     tc.tile_pool(name="qtp", bufs=4) as qtp, \
                     tc.tile_pool(name="ptp", bufs=6) as ptp, \
                     tc.tile_pool(name="tmpp", bufs=3) as tmpp, \
                     tc.tile_pool(name="bcp", bufs=2) as bcp, \
                     tc.tile_pool(name="rcp", bufs=3) as rcp, \
                     tc.tile_pool(name="ytnp", bufs=1) as ytnp, \
                     tc.tile_pool(name="obp", bufs=3) as obp, \
                     tc.tile_pool(name="psT", bufs=3, space="PSUM") as psT, \
                     tc.tile_pool(name="psY", bufs=2, space="PSUM") as psY, \
                     tc.tile_pool(name="psS", bufs=1, space="PSUM") as psS, \
                     tc.tile_pool(name="psO", bufs=2, space="PSUM") as psO:

                    kt_sb = []
                    for h in range(HPC):
                        kt = ktp.tile([P, S], MMDT, name=f"kt{h}", tag=f"kt{h}")
                        for n in range(NR):
                            nc.sync.dma_start(kt[:, n * 512:(n + 1) * 512],
                                              kT_d[h, n])
                        kt_sb.append(kt)

                    for r in range(NR):
                        ytn_tiles = []
                        for h in range(HPC):
                            qt = qtp.tile([P, STQ], MMDT,
                                          name=f"qt{r}_{h}", tag="qt")
                            nc.sync.dma_start(qt, qT_d[h, r])
                            nj = 4 * (r + 1)
                            yt_ps = psY.tile([P, STQ], F32,
                                             name=f"yt{r}_{h}", tag="yt")
                            sm_ps = psS.tile([1, STQ], F32,
                                             name=f"sm{r}_{h}", tag="sm")
                            for j in range(nj):
                                st_ps = psT.tile([P, STQ], F32,
                                                 name=f"st{r}_{h}_{j}", tag="st")
                                mm(st_ps, lhsT=kt_sb[h][:, j * P:(j + 1) * P],
                                   rhs=qt, start=True, stop=True)
                                pt = ptp.tile([P, STQ], MMDT,
                                              name=f"pt{r}_{h}_{j}", tag="pt")
                                if j >= nj - 4:
                                    tmp = tmpp.tile([P, STQ], F32,
                                                    name=f"tm{r}_{h}_{j}",
                                                    tag="tmp")
                                    nc.vector.tensor_add(
                                        tmp, st_ps, masks[j - (nj - 4)])
                                    nc.scalar.activation(pt, tmp, Exp,
                                                         scale=NORM)
                                else:
                                    nc.scalar.activation(pt, st_ps, Exp,
                                                         scale=NORM)
                                vt = vsp.tile([P, D], MMDT,
                                              name=f"v{r}_{h}_{j}", tag="v")
                                nc.sync.dma_start(vt, v_d[h, j])
                                mm(sm_ps, lhsT=ones_col, rhs=pt,
                                   start=(j == 0), stop=(j == nj - 1))
                                mm(yt_ps, lhsT=vt, rhs=pt,
                                   start=(j == 0), stop=(j == nj - 1))
                            rc = rcp.tile([1, STQ], F32,
                                          name=f"rc{r}_{h}", tag="rc")
                            nc.vector.reciprocal(rc, sm_ps)
                            rcr = rcp.tile([1, STQ], MMDT,
                                           name=f"rcr{r}_{h}", tag="rcr")
                            nc.scalar.copy(rcr, rc)
                            bc_ps = psT.tile([P, STQ], F32,
                                             name=f"bcp{r}_{h}", tag="st")
                            mm(bc_ps, lhsT=ones_row, rhs=rcr,
                               start=True, stop=True)
                            bc_sb = bcp.tile([P, STQ], F32,
                                             name=f"bc{r}_{h}", tag="bc")
                            nc.scalar.copy(bc_sb, bc_ps)
                            ytn = ytnp.tile([P, STQ], MMDT,
                                            name=f"ytn{r}_{h}", tag=f"ytn{h}")
                            nc.vector.tensor_mul(ytn, yt_ps, bc_sb)
                            ytn_tiles.append(ytn)

                        for n in range(NN):
                            wpt = []
                            for h in range(HPC):
                                wp_t = wpp.tile([P, 512], MMDT,
                                                name=f"wp{r}_{n}_{h}", tag="wp")
                                nc.sync.dma_start(wp_t, wp_d[h, n])
                                wpt.append(wp_t)
                            for qq in range(4):
                                o_ps = psO.tile([P, 512], F32,
                                                name=f"o{r}_{n}_{qq}", tag="o")
                                for h in range(HPC):
                                    mm(o_ps,
                                       lhsT=ytn_tiles[h][:, qq * P:(qq + 1) * P],
                                       rhs=wpt[h],
                                       start=(h == 0), stop=(h == HPC - 1))
                                o_sb = obp.tile([P, 512], F32,
                                                name=f"ob{r}_{n}_{qq}",
                                                tag="ob")
                                nc.scalar.copy(o_sb, o_ps)
                                nc.sync.dma_start(
                                    o_d[r * STQ + qq * P:
                                        r * STQ + (qq + 1) * P,
                                        n * 512:(n + 1) * 512], o_sb)

    nc.compile()
    return nc


_NC = None


def _get_program():
    global _NC
    if _NC is None:
        _NC = _build_program()
    return _NC


def _maybe_install_trace_shim():
    """Provide antenv.axon_hooks (NTFF profiling) if the image lacks it."""
    import types
    if "antenv.axon_hooks" in sys.modules:
        return
    try:
        from trn_agent_boot.trn_boot import _ntff_profile_via_ctypes
        hook = _ntff_profile_via_ctypes("/opt/axon/libaxon_pjrt.so")
    except Exception:
        return
    mod = types.ModuleType("antenv.axon_hooks")
    mod.get_axon_ntff_profile_hook = lambda: hook
    mod.set_axon_ntff_profile_hook = lambda h: None
    sys.modules["antenv.axon_hooks"] = mod


def kernel(x, W_attn, b_attn, W_proj, b_proj):
    x = np.ascontiguousarray(np.asarray(x, dtype=np.float32))
    W_attn = np.ascontiguousarray(np.asarray(W_attn, dtype=np.float32))
    b_attn = np.ascontiguousarray(np.asarray(b_attn, dtype=np.float32))
    W_proj = np.ascontiguousarray(np.asarray(W_proj, dtype=np.float32))
    b_proj = np.ascontiguousarray(np.asarray(b_proj, dtype=np.float32))

    nc = _get_program()

    # per-parity weight shards (heads p*8 .. p*8+8), packed per DMA layouts
    shards = []
    for p in range(2):
        cs = slice(p * HPC * D, (p + 1) * HPC * D)
        # [E, HPC*D] -> [HPC, NE, P, D]
        def pack_hcpd(w):
            return np.ascontiguousarray(
                w.reshape(NE, P, HPC, D).transpose(2, 0, 1, 3))

        wq = W_attn[:, 0 * E:1 * E][:, cs]
        wk = W_attn[:, 1 * E:2 * E][:, cs]
        wv = W_attn[:, 2 * E:3 * E][:, cs]
        wp = W_proj[cs, :]
        shards.append({
            "wq": pack_hcpd(wq),
            "wk": pack_hcpd(wk),
            # [E, 1024] -> [2, NE, P, 512]
            "wv": np.ascontiguousarray(
                wv.reshape(NE, P, 2, 512).transpose(2, 0, 1, 3)),
            # [1024, E] -> [HPC, NN, P, 512]
            "wp": np.ascontiguousarray(
                wp.reshape(HPC, P, NN, 512).transpose(0, 2, 1, 3)),
            "bq": np.ascontiguousarray(
                b_attn[0 * E:1 * E][cs].reshape(HPC, D).T),
        })
    xTs = [np.ascontiguousarray(x[b].T) for b in range(B)]

    in_maps = []
    for core in range(8):
        b, p = core // 2, core % 2
        m = {"xT": xTs[b]}
        m.update(shards[p])
        in_maps.append(m)

    trace = bool(os.environ.get("CK_TRACE"))
    if trace:
        _maybe_install_trace_shim()
    res = run_bass_kernel_spmd(nc, in_maps, core_ids=list(range(8)),
                               trace=trace)
    if trace:
        kernel.last_exec_time_ns = res.exec_time_ns
        kernel.last_trace = res.instructions_and_trace

    # ------- host-side gather -------
    b_k = b_attn[1 * E:2 * E]
    b_v = b_attn[2 * E:3 * E]

    out = np.empty((B, S, E), dtype=np.float32)
    k_full = np.empty((B, H, S, D), dtype=np.float32)
    v_full = np.empty((B, H, S, D), dtype=np.float32)
    bias_out = (b_v @ W_proj + b_proj).astype(np.float32)

    for bi in range(B):
        r0, r1 = res.results[2 * bi], res.results[2 * bi + 1]
        out[bi] = r0["out_p"] + r1["out_p"] + bias_out[None, :]
        for p, r in ((0, r0), (1, r1)):
            # kT_out: [HPC, NR, P, 512] -> k[h] = [S, D]
            kt = r["kT_out"]
            vv = r["v_out"]
            for j in range(HPC):
                h = p * HPC + j
                # [NR, P(d), 512(s)] -> [d, NR*512] -> [s, d]
                k_full[bi, h] = (kt[j].transpose(1, 0, 2).reshape(D, S).T
                                 + b_k[h * D:(h + 1) * D][None, :])
                # [NS, P(s), D] -> [S, D]
                v_full[bi, h] = (vv[j].reshape(S, D)
                                 + b_v[h * D:(h + 1) * D][None, :])

    return out, k_full, v_full


# revision 6
# speedup vs baseline: 1.2593x; 1.1141x over previous
"""Causal self-attention (B=4, S=2048, E=2048, H=16, D=128) on 8 TRN2 cores.

Sharding: batch (4-way) x head-halves (2-way) -> 8 cores.
Core c handles batch b = c//2, heads p*8..p*8+8 where p = c%2.

Per-core kernel (single NEFF, SPMD):
  Phase 1: QKV projection.  x^T resident in SBUF [E,S]; per head computes
           q^T, k^T in [D,S] layout (matmul lhsT = W columns, rhs = x^T) and
           v in [S,D] layout (lhsT = x^T tile, rhs = W_v columns).
           q^T -> DRAM scratch, k^T -> output, v -> output.
  Phase 2: attention + proj in one pass; k^T resident for all 8 heads,
           v / Wproj / q^T streamed.
           Scores computed transposed: ST[k,q] = (kT chunk) lhsT vs qT rhs.
           exp on scalar engine (no max subtraction needed: scores ~ N(0,1)),
           row sums via ones-vector matmul, y^T via lhsT=v rhs=P^T,
           normalization via PE outer-product broadcast of 1/sums,
           out = y @ Wproj accumulated over all 8 heads in PSUM.
Host: shards inputs, sums the 2 partial outs per batch, fixes up k/v biases.

All DRAM tensors are laid out so every DMA transfer is a single contiguous
block (weights pre-packed on host into [head, chunk, 128, width] form).

Matmul inputs use float32r (single-pass PE, mantissa rounded to 12 bits)
unless CK_MM_DT=f32 (2-pass full fp32, ~4x slower).
"""

import math
import os
import sys

for _p in ("/opt/trn_rl_repo",):
    if os.path.isdir(_p) and _p not in sys.path:
        sys.path.append(_p)

import numpy as np

import concourse.bacc as bacc
import concourse.mybir as mybir
import concourse.tile as tile
from concourse.bass_utils import run_bass_kernel_spmd

B, S, E, H = 4, 2048, 2048, 16
D = E // H            # 128
P = 128               # partitions
HPC = H // 2          # heads per core = 8
NE = E // P           # 16 e-chunks
NS = S // P           # 16 s-chunks
STQ = 512             # phase-2 q tile width
NR = S // STQ         # 4 q tiles
NN = E // 512         # 4 output col chunks
NORM = 1.0 / math.sqrt(D)
NEG = -1.0e30

F32 = mybir.dt.float32
F32R = mybir.dt.float32r

# matmul input dtype: float32r = full-rate single-pass (12-bit mantissa),
# float32 = 4x slower 2-pass full precision.
MMDT = {"f32": F32, "f32r": F32R}[os.environ.get("CK_MM_DT", "f32r")]


def _build_program():
    nc = bacc.Bacc("TRN2", target_bir_lowering=False, debug=False)
    Exp = mybir.ActivationFunctionType.Exp
    mm = nc.tensor.matmul

    with tile.TileContext(nc) as tc:
        with tc.tile_pool(name="dram", bufs=1, space="DRAM") as dram:
            def din(name, shape, dt=MMDT):
                return dram.tile(shape, dt, kind="ExternalInput", name=name,
                                 uniquify=False)

            def dout(name, shape, dt=F32):
                return dram.tile(shape, dt, kind="ExternalOutput", name=name,
                                 uniquify=False)

            xT_d = din("xT", [E, S])
            wq_d = din("wq", [HPC, NE, P, D])
            wk_d = din("wk", [HPC, NE, P, D])
            wv_d = din("wv", [2, NE, P, 512])
            wp_d = din("wp", [HPC, NN, P, 512])
            bq_d = din("bq", [P, HPC], F32)
            # k^T / v outputs carry matmul-rounded values (valid fp32 bits)
            kT_d = dout("kT_out", [HPC, NR, P, 512], MMDT)
            v_d = dout("v_out", [HPC, NS, P, D], MMDT)
            o_d = dout("out_p", [S, E])
            qT_d = dram.tile([HPC, NR, P, 512], MMDT, kind="Internal",
                             name="qT_s", uniquify=False)

            # ---------------- constants ----------------
            with tc.tile_pool(name="const", bufs=1) as cp:
                masks = []
                for m in range(4):
                    mk = cp.tile([P, STQ], F32, name=f"mask{m}", tag=f"mask{m}")
                    nc.gpsimd.memset(mk, 0.0)
                    # ST layout [k, q]: keep where q - k - 128*m >= 0
                    nc.gpsimd.affine_select(
                        out=mk, in_=mk,
                        compare_op=mybir.AluOpType.is_ge,
                        fill=NEG, base=-P * m,
                        channel_multiplier=-1, pattern=[[1, STQ]],
                    )
                    masks.append(mk)
                ones_col_f = cp.tile([P, 1], F32, name="ones_col_f", tag="oncf")
                nc.gpsimd.memset(ones_col_f, 1.0)
                ones_col = cp.tile([P, 1], MMDT, name="ones_col", tag="onc")
                nc.scalar.copy(ones_col, ones_col_f)
                ones_row_f = cp.tile([1, P], F32, name="ones_row_f", tag="onrf")
                nc.gpsimd.memset(ones_row_f, 1.0)
                ones_row = cp.tile([1, P], MMDT, name="ones_row", tag="onr")
                nc.scalar.copy(ones_row, ones_row_f)
                bq_sb = cp.tile([P, HPC], F32, name="bq_sb", tag="bq")
                nc.sync.dma_start(bq_sb, bq_d)

                # ---------------- phase 1: QKV ----------------
                with tc.tile_pool(name="p1", bufs=1) as p1, \
                     tc.tile_pool(name="wstream", bufs=8) as ws, \
                     tc.tile_pool(name="wvres", bufs=NE) as wvp, \
                     tc.tile_pool(name="ev", bufs=6) as evp:

                    xt = p1.tile([P, NE, S], MMDT, name="xt", tag="xt")
                    for c in range(NE):
                        nc.sync.dma_start(xt[:, c, :], xT_d[c * P:(c + 1) * P, :])

                    # q^T and k^T per head, interleaved per e-chunk
                    with tc.tile_pool(name="psA", bufs=8, space="PSUM") as psA:
                        for h in range(HPC):
                            psq = [psA.tile([P, 512], F32,
                                            name=f"ps_q{h}_{n}", tag="ps")
                                   for n in range(4)]
                            psk = [psA.tile([P, 512], F32,
                                            name=f"ps_k{h}_{n}", tag="ps")
                                   for n in range(4)]
                            for c in range(NE):
                                wtq = ws.tile([P, D], MMDT,
                                              name=f"w_q{h}_{c}", tag="w")
                                nc.sync.dma_start(wtq, wq_d[h, c])
                                wtk = ws.tile([P, D], MMDT,
                                              name=f"w_k{h}_{c}", tag="w")
                                nc.sync.dma_start(wtk, wk_d[h, c])
                                for n in range(4):
                                    mm(psq[n], lhsT=wtq,
                                       rhs=xt[:, c, n * 512:(n + 1) * 512],
                                       start=(c == 0), stop=(c == NE - 1))
                                for n in range(4):
                                    mm(psk[n], lhsT=wtk,
                                       rhs=xt[:, c, n * 512:(n + 1) * 512],
                                       start=(c == 0), stop=(c == NE - 1))
                            for n in range(4):
                                ev = evp.tile([P, 512], MMDT,
                                              name=f"ev_q{h}_{n}", tag="ev")
                                nc.scalar.activation(
                                    ev, psq[n],
                                    mybir.ActivationFunctionType.Identity,
                                    bias=bq_sb[:, h:h + 1], scale=1.0)
                                nc.sync.dma_start(qT_d[h, n], ev)
                                ev2 = evp.tile([P, 512], MMDT,
                                               name=f"ev_k{h}_{n}", tag="ev")
                                nc.scalar.copy(ev2, psk[n])
                                nc.sync.dma_start(kT_d[h, n], ev2)

                    # v in [S, 4*D] groups of 4 heads
                    with tc.tile_pool(name="psV", bufs=4, space="PSUM") as psV:
                        for g4 in range(2):
                            wvt = []
                            for c in range(NE):
                                wvc = wvp.tile([P, 512], MMDT,
                                               name=f"wv_{g4}_{c}", tag="wv")
                                nc.sync.dma_start(wvc, wv_d[g4, c])
                                wvt.append(wvc)
                            for st in range(NS):
                                pv = psV.tile([P, 512], F32,
                                              name=f"pv_{g4}_{st}", tag="pv")
                                for c in range(NE):
                                    mm(pv, lhsT=xt[:, c, st * P:(st + 1) * P],
                                       rhs=wvt[c],
                                       start=(c == 0), stop=(c == NE - 1))
                                ev = evp.tile([P, 512], MMDT,
                                              name=f"evv_{g4}_{st}", tag="ev")
                                nc.scalar.copy(ev, pv)
                                for hh in range(4):
                                    h = g4 * 4 + hh
                                    nc.sync.dma_start(
                                        v_d[h, st], ev[:, hh * D:(hh + 1) * D])

                # ---------------- phase 2: attention + proj ----------------
                with tc.tile_pool(name="ktp", bufs=1) as ktp, \
                     tc.tile_pool(name="vsp", bufs=8) as vsp, \
                     tc.tile_pool(name="wpp", bufs=16) as wpp, \
                     tc.tile_pool(name="qtp", bufs=4) as qtp, \
                     tc.tile_pool(name="ptp", bufs=6) as ptp, \
                     tc.tile_pool(name="tmpp", bufs=3) as tmpp, \
                     tc.tile_pool(name="bcp", bufs=2) as bcp, \
                     tc.tile_pool(name="rcp", bufs=3) as rcp, \
                     tc.tile_pool(name="ytnp", bufs=1) as ytnp, \
                     tc.tile_pool(name="obp", bufs=3) as obp, \
                     tc.tile_pool(name="psX", bufs=4, space="PSUM") as psX, \
                     tc.tile_pool(name="psY", bufs=2, space="PSUM") as psY, \
                     tc.tile_pool(name="psS", bufs=2, space="PSUM") as psS:

                    kt_sb = [ktp.tile([P, S], MMDT, name=f"kt{h}", tag=f"kt{h}")
                             for h in range(HPC)]
                    for n in range(NR):
                        for h in range(HPC):
                            nc.sync.dma_start(
                                kt_sb[h][:, n * 512:(n + 1) * 512], kT_d[h, n])

                    for r in range(NR):
                        nj = 4 * (r + 1)
                        ytn_tiles = {}
                        pending = []  # heads awaiting normalization
                        sm_of, yt_of = {}, {}

                        def finalize(h):
                            rc = rcp.tile([1, STQ], F32,
                                          name=f"rc_{h}", tag="rc")
                            nc.vector.reciprocal(rc, sm_of[h])
                            rcr = rcp.tile([1, STQ], MMDT,
                                           name=f"rcr_{h}", tag="rcr")
                            nc.scalar.copy(rcr, rc)
                            bc_ps = psX.tile([P, STQ], F32,
                                             name=f"bcp_{h}", tag="x")
                            mm(bc_ps, lhsT=ones_row, rhs=rcr,
                               start=True, stop=True)
                            bc_sb = bcp.tile([P, STQ], F32,
                                             name=f"bc_{h}", tag="bc")
                            nc.scalar.copy(bc_sb, bc_ps)
                            ytn = ytnp.tile([P, STQ], MMDT,
                                            name=f"ytn_{h}", tag=f"ytn{h}")
                            nc.vector.tensor_mul(ytn, yt_of[h], bc_sb)
                            ytn_tiles[h] = ytn

                        for h in range(HPC):
                            qt = qtp.tile([P, STQ], MMDT,
                                          name=f"qt{r}_{h}", tag="qt")
                            nc.sync.dma_start(qt, qT_d[h, r])
                            yt_ps = psY.tile([P, STQ], F32,
                                             name=f"yt{r}_{h}", tag="yt")
                            sm_ps = psS.tile([1, STQ], F32,
                                             name=f"sm{r}_{h}", tag="sm")
                            sm_of[h], yt_of[h] = sm_ps, yt_ps
                            for j in range(nj):
                                st_ps = psX.tile([P, STQ], F32,
                                                 name=f"st{r}_{h}_{j}", tag="x")
                                mm(st_ps, lhsT=kt_sb[h][:, j * P:(j + 1) * P],
                                   rhs=qt, start=True, stop=True)
                                pt = ptp.tile([P, STQ], MMDT,
                                              name=f"pt{r}_{h}_{j}", tag="pt")
                                if j >= nj - 4:
                                    tmp = tmpp.tile([P, STQ], F32,
                                                    name=f"tm{r}_{h}_{j}",
                                                    tag="tmp")
                                    nc.vector.tensor_add(
                                        tmp, st_ps, masks[j - (nj - 4)])
                                    nc.scalar.activation(pt, tmp, Exp,
                                                         scale=NORM)
                                else:
                                    nc.scalar.activation(pt, st_ps, Exp,
                                                         scale=NORM)
                                vt = vsp.tile([P, D], MMDT,
                                              name=f"v{r}_{h}_{j}", tag="v")
                                nc.sync.dma_start(vt, v_d[h, j])
                                mm(sm_ps, lhsT=ones_col, rhs=pt,
                                   start=(j == 0), stop=(j == nj - 1))
                                mm(yt_ps, lhsT=vt, rhs=pt,
                                   start=(j == 0), stop=(j == nj - 1))
                            pending.append(h)
                            if len(pending) > 1:
                                finalize(pending.pop(0))
                        finalize(pending.pop(0))

                        for n in range(NN):
                            wpt = []
                            for h in range(HPC):
                                wp_t = wpp.tile([P, 512], MMDT,
                                                name=f"wp{r}_{n}_{h}", tag="wp")
                                nc.sync.dma_start(wp_t, wp_d[h, n])
                                wpt.append(wp_t)
                            for qq in range(4):
                                o_ps = psX.tile([P, 512], F32,
                                                name=f"o{r}_{n}_{qq}", tag="x")
                                for h in range(HPC):
                                    mm(o_ps,
                                       lhsT=ytn_tiles[h][:, qq * P:(qq + 1) * P],
                                       rhs=wpt[h],
                                       start=(h == 0), stop=(h == HPC - 1))
                                o_sb = obp.tile([P, 512], F32,
                                                name=f"ob{r}_{n}_{qq}",
                                                tag="ob")
                                nc.scalar.copy(o_sb, o_ps)
                                nc.sync.dma_start(
                                    o_d[r * STQ + qq * P:
                                        r * STQ + (qq + 1) * P,
                                        n * 512:(n + 1) * 512], o_sb)

    nc.compile()
    return nc


_NC = None


def _get_program():
    global _NC
    if _NC is None:
        _NC = _build_program()
    return _NC


def _maybe_install_trace_shim():
    """Provide antenv.axon_hooks (NTFF profiling) if the image lacks it."""
    import types
    if "antenv.axon_hooks" in sys.modules:
        return
    try:
        from trn_agent_boot.trn_boot import _ntff_profile_via_ctypes
        hook = _ntff_profile_via_ctypes("/opt/axon/libaxon_pjrt.so")
    except Exception:
        return
    mod = types.ModuleType("antenv.axon_hooks")
    mod.get_axon_ntff_profile_hook = lambda: hook
    mod.set_axon_ntff_profile_hook = lambda h: None
    sys.modules["antenv.axon_hooks"] = mod


def kernel(x, W_attn, b_attn, W_proj, b_proj):
    x = np.ascontiguousarray(np.asarray(x, dtype=np.float32))
    W_attn = np.ascontiguousarray(np.asarray(W_attn, dtype=np.float32))
    b_attn = np.ascontiguousarray(np.asarray(b_attn, dtype=np.float32))
    W_proj = np.ascontiguousarray(np.asarray(W_proj, dtype=np.float32))
    b_proj = np.ascontiguousarray(np.asarray(b_proj, dtype=np.float32))

    nc = _get_program()

    # per-parity weight shards (heads p*8 .. p*8+8), packed per DMA layouts
    shards = []
    for p in range(2):
        cs = slice(p * HPC * D, (p + 1) * HPC * D)
        # [E, HPC*D] -> [HPC, NE, P, D]
        def pack_hcpd(w):
            return np.ascontiguousarray(
                w.reshape(NE, P, HPC, D).transpose(2, 0, 1, 3))

        wq = W_attn[:, 0 * E:1 * E][:, cs]
        wk = W_attn[:, 1 * E:2 * E][:, cs]
        wv = W_attn[:, 2 * E:3 * E][:, cs]
        wp = W_proj[cs, :]
        shards.append({
            "wq": pack_hcpd(wq),
            "wk": pack_hcpd(wk),
            # [E, 1024] -> [2, NE, P, 512]
            "wv": np.ascontiguousarray(
                wv.reshape(NE, P, 2, 512).transpose(2, 0, 1, 3)),
            # [1024, E] -> [HPC, NN, P, 512]
            "wp": np.ascontiguousarray(
                wp.reshape(HPC, P, NN, 512).transpose(0, 2, 1, 3)),
            "bq": np.ascontiguousarray(
                b_attn[0 * E:1 * E][cs].reshape(HPC, D).T),
        })
    xTs = [np.ascontiguousarray(x[b].T) for b in range(B)]

    in_maps = []
    for core in range(8):
        b, p = core // 2, core % 2
        m = {"xT": xTs[b]}
        m.update(shards[p])
        in_maps.append(m)

    trace = bool(os.environ.get("CK_TRACE"))
    if trace:
        _maybe_install_trace_shim()
    res = run_bass_kernel_spmd(nc, in_maps, core_ids=list(range(8)),
                               trace=trace)
    if trace:
        kernel.last_exec_time_ns = res.exec_time_ns
        kernel.last_trace = res.instructions_and_trace

    # ------- host-side gather -------
    b_k = b_attn[1 * E:2 * E]
    b_v = b_attn[2 * E:3 * E]

    out = np.empty((B, S, E), dtype=np.float32)
    k_full = np.empty((B, H, S, D), dtype=np.float32)
    v_full = np.empty((B, H, S, D), dtype=np.float32)
    bias_out = (b_v @ W_proj + b_proj).astype(np.float32)

    for bi in range(B):
        r0, r1 = res.results[2 * bi], res.results[2 * bi + 1]
        out[bi] = r0["out_p"] + r1["out_p"] + bias_out[None, :]
        for p, r in ((0, r0), (1, r1)):
            # kT_out: [HPC, NR, P, 512] -> k[h] = [S, D]
            kt = r["kT_out"]
            vv = r["v_out"]
            for j in range(HPC):
                h = p * HPC + j
                # [NR, P(d), 512(s)] -> [d, NR*512] -> [s, d]
                k_full[bi, h] = (kt[j].transpose(1, 0, 2).reshape(D, S).T
                                 + b_k[h * D:(h + 1) * D][None, :])
                # [NS, P(s), D] -> [S, D]
                v_full[bi, h] = (vv[j].reshape(S, D)
                                 + b_v[h * D:(h + 1) * D][None, :])

    return out, k_full, v_full
